# revision 3
# baseline (speedup 1.0000x reference)
"""DeepAttnMISL segment-reduce kernel for 8 TRN2 NeuronCores.

Strategy (per sharding hint): shard the N=200000 patches across the 8 cores.
Each core computes phi = relu(X_shard @ W_phi.T + b_phi) and per-cluster
partial sums via a one-hot matmul accumulated in PSUM. The tiny [10, 256]
per-core partial sums are reduced on the host, and the (tiny) attention
pooling + output head run on the host in fp32.

Device-side layout choices:
  - The d_in contraction dim must land on SBUF partitions, so the host
    pre-permutes X into [128, blocks*kc*blkn] per core (fp32 hardware
    DMA-transpose is not supported, and multi-chunk DMA APs hang TRN2, so
    every DMA here is a plain 2D row-contiguous transfer).
  - Each core gets a 25088-row (196x128) window of X; windows overlap so all
    cores run the identical SPMD program. The host-built one-hot matrix is
    zeroed outside each core's owned range, so overlap rows contribute
    nothing to the cluster sums.
  - Cluster counts depend only on cluster_id and are computed on the host.
"""

import numpy as np

import concourse.mybir as mybir
import concourse.tile as tile
from concourse import bacc
from concourse.bass_utils import run_bass_kernel_spmd

N = 200000
D_IN = 1024
D_HID = 256
NUM_CLUSTERS = 10
NCORES = 8
P = 128
KC = D_IN // P          # 8 contraction chunks
NP = 25088              # padded patches per core (196 * 128)
TILES = NP // P         # 196 patch tiles per core
TPB = 14                # patch tiles per DMA block
BLOCKS = TILES // TPB   # 14 blocks
BLKN = TPB * P          # 1792 patches per block
OWN = N // NCORES       # 25000 owned rows per core
XCOLS = KC * BLKN       # free-dim columns per block in the device X layout

_CACHE = {}


def _build():
    if "nc" in _CACHE:
        return _CACHE["nc"]
    f32 = mybir.dt.float32
    nc = bacc.Bacc("TRN2", target_bir_lowering=False, debug=False, num_devices=NCORES)

    xt_d = nc.dram_tensor("xt", [P, BLOCKS * XCOLS], f32, kind="ExternalInput").ap()
    wt_d = nc.dram_tensor("wt", [P, KC * D_HID], f32, kind="ExternalInput").ap()
    oh_d = nc.dram_tensor("oh", [P, TILES * NUM_CLUSTERS], f32, kind="ExternalInput").ap()
    bb_d = nc.dram_tensor("bb", [P, D_HID], f32, kind="ExternalInput").ap()
    sums_d = nc.dram_tensor("sums", [NUM_CLUSTERS, D_HID], f32, kind="ExternalOutput").ap()

    with tile.TileContext(nc) as tc:
        with (
            tc.tile_pool(name="consts", bufs=1) as cpool,
            tc.tile_pool(name="xt", bufs=2) as xpool,
            tc.tile_pool(name="phi", bufs=4) as fpool,
            tc.tile_pool(name="psum", bufs=4, space="PSUM") as ppool,
            tc.tile_pool(name="acc", bufs=1, space="PSUM") as apool,
        ):
            wt_sb = cpool.tile([P, KC * D_HID], f32)
            nc.sync.dma_start(out=wt_sb, in_=wt_d)
            oh_sb = cpool.tile([P, TILES * NUM_CLUSTERS], f32)
            nc.sync.dma_start(out=oh_sb, in_=oh_d)
            bb_sb = cpool.tile([P, D_HID], f32)
            nc.sync.dma_start(out=bb_sb, in_=bb_d)

            sums_ps = apool.tile([NUM_CLUSTERS, D_HID], f32)

            for blk in range(BLOCKS):
                xt_sb = xpool.tile([P, XCOLS], f32)
                nc.sync.dma_start(out=xt_sb, in_=xt_d[:, blk * XCOLS:(blk + 1) * XCOLS])
                for j in range(TPB):
                    t = blk * TPB + j
                    phi_ps = ppool.tile([P, D_HID], f32)
                    for k in range(KC):
                        nc.tensor.matmul(
                            phi_ps,
                            xt_sb[:, k * BLKN + j * P: k * BLKN + (j + 1) * P],
                            wt_sb[:, k * D_HID:(k + 1) * D_HID],
                            start=(k == 0),
                            stop=(k == KC - 1),
                        )
                    tmp_sb = fpool.tile([P, D_HID], f32, tag="tmp")
                    nc.vector.tensor_add(tmp_sb, phi_ps, bb_sb)
                    phi_sb = fpool.tile([P, D_HID], f32, tag="phi")
                    nc.scalar.activation(phi_sb, tmp_sb, mybir.ActivationFunctionType.Relu)
                    nc.tensor.matmul(
                        sums_ps,
                        oh_sb[:, t * NUM_CLUSTERS:(t + 1) * NUM_CLUSTERS],
                        phi_sb,
                        start=(t == 0),
                        stop=(t == TILES - 1),
                    )

            out_sb = cpool.tile([NUM_CLUSTERS, D_HID], f32)
            nc.vector.tensor_copy(out_sb, sums_ps)
            nc.sync.dma_start(out=sums_d, in_=out_sb)

    nc.compile()
    _CACHE["nc"] = nc
    return nc


def _shard_layout(x2, win):
    """Device X layout for one core: [P, BLOCKS*KC*BLKN] where
    out[p, (b*KC + kc)*BLKN + n] = x2[win + b*BLKN + n, kc*P + p]."""
    xw = x2[win:win + NP]                                  # [NP, D_IN]
    xr = xw.reshape(BLOCKS, BLKN, KC, P)
    out = np.empty((P, BLOCKS, KC, BLKN), np.float32)
    for b in range(BLOCKS):                                # blocked for cache
        out[:, b] = xr[b].transpose(2, 1, 0)               # [P, KC, BLKN]
    return out.reshape(P, BLOCKS * XCOLS)


def _prepare_in_maps(X, cluster_id, W_phi, b_phi):
    x2 = np.ascontiguousarray(np.asarray(X, np.float32)[0])
    cid = np.asarray(cluster_id).astype(np.int64)

    onehot = (cid[:, None] == np.arange(NUM_CLUSTERS)[None, :]).astype(np.float32)

    wp = np.asarray(W_phi, np.float32)                     # [256, 1024]
    # wt[p, kc*256 + d] = W_phi[d, kc*128 + p]
    wt = np.ascontiguousarray(wp.T.reshape(KC, P, D_HID).transpose(1, 0, 2).reshape(P, KC * D_HID))
    bbc = np.ascontiguousarray(
        np.broadcast_to(np.asarray(b_phi, np.float32)[None, :], (P, D_HID))
    )

    in_maps = []
    for c in range(NCORES):
        win = min(c * OWN, N - NP)          # window start (core 7 shifts back)
        own_lo = c * OWN - win              # owned range within window
        own_hi = own_lo + OWN
        xt = _shard_layout(x2, win)
        oh = np.zeros((NP, NUM_CLUSTERS), np.float32)
        oh[own_lo:own_hi] = onehot[win + own_lo: win + own_hi]
        # device layout: [128, TILES*10], entry [p, t*10+c] = onehot[t*128+p, c]
        ohr = np.ascontiguousarray(
            oh.reshape(TILES, P, NUM_CLUSTERS).transpose(1, 0, 2).reshape(P, TILES * NUM_CLUSTERS)
        )
        in_maps.append({"xt": xt, "wt": wt, "oh": ohr, "bb": bbc})
    return in_maps


def kernel(X, cluster_id, W_phi, b_phi, W1, b1, Wa, ba, Wb, bb, Wc, bc, Wo, bo):
    cid = np.asarray(cluster_id).astype(np.int64)
    in_maps = _prepare_in_maps(X, cluster_id, W_phi, b_phi)

    nc = _build()
    res = run_bass_kernel_spmd(nc, in_maps, list(range(NCORES)))

    sums = np.zeros((NUM_CLUSTERS, D_HID), np.float32)
    for c in range(NCORES):
        sums += res.results[c]["sums"]

    counts = np.bincount(cid, minlength=NUM_CLUSTERS).astype(np.float32)

    # tiny attention-pooling + output head, fp32 on host (matches reference)
    h = np.where(counts[:, None] > 0, sums / np.maximum(counts, 1.0)[:, None], 0.0).astype(np.float32)
    h1 = np.maximum(h @ np.asarray(W1, np.float32).T + b1, 0.0).astype(np.float32)
    a = np.tanh(h1 @ np.asarray(Wa, np.float32).T + ba).astype(np.float32)
    g = (1.0 / (1.0 + np.exp(-(h1 @ np.asarray(Wb, np.float32).T + bb)))).astype(np.float32)
    scores = ((a * g) @ np.asarray(Wc, np.float32).T + bc).astype(np.float32)  # [10, 1]
    s = scores.T  # [1, 10]
    e = np.exp(s - s.max(axis=-1, keepdims=True))
    A = (e / e.sum(axis=-1, keepdims=True)).astype(np.float32)
    H = (A @ h1).astype(np.float32)
    out = (H @ np.asarray(Wo, np.float32).T + bo).astype(np.float32)
    return out


# revision 11
# speedup vs baseline: 2.5225x; 2.5225x over previous
"""DeepAttnMISL segment-reduce kernel for 8 TRN2 NeuronCores.

Strategy (per sharding hint): shard the N=200000 patches across the 8 cores.
Each core computes phi = relu(X_shard @ W_phi.T + b_phi) and per-cluster
partial sums via a one-hot matmul accumulated in PSUM. The tiny [10, 256]
per-core partial sums are reduced on the host, and the (tiny) attention
pooling + output head run on the host in fp32.

Device-side layout choices:
  - The d_in contraction dim must land on SBUF partitions, so the host
    pre-permutes X into [128, blocks*kc*blkn] per core (fp32 hardware
    DMA-transpose is not supported, and multi-chunk DMA APs hang TRN2, so
    every DMA here is a plain 2D row-contiguous transfer).
  - Each core gets a 25088-row (196x128) window of X; windows overlap so all
    cores run the identical SPMD program. The host-built one-hot matrix is
    zeroed outside each core's owned range, so overlap rows contribute
    nothing to the cluster sums.
  - Cluster counts depend only on cluster_id and are computed on the host.
"""

import numpy as np

import concourse.mybir as mybir
import concourse.tile as tile
from concourse import bacc
from concourse.bass_utils import run_bass_kernel_spmd

N = 200000
D_IN = 1024
D_HID = 256
NUM_CLUSTERS = 10
NCORES = 8
P = 128
KC = D_IN // P          # 8 contraction chunks
NP = 25088              # padded patches per core (196 * 128)
TILES = NP // P         # 196 patch tiles per core
TPB = 14                # patch tiles per DMA block
BLOCKS = TILES // TPB   # 14 blocks
BLKN = TPB * P          # 1792 patches per block
OWN = N // NCORES       # 25000 owned rows per core
XCOLS = KC * BLKN       # free-dim columns per block in the device X layout

_CACHE = {}


def _build():
    if "nc" in _CACHE:
        return _CACHE["nc"]
    f32 = mybir.dt.float32
    f32r = mybir.dt.float32r
    nc = bacc.Bacc("TRN2", target_bir_lowering=False, debug=False, num_devices=NCORES)

    xt_d = nc.dram_tensor("xt", [P, BLOCKS * XCOLS], f32r, kind="ExternalInput").ap()
    wt_d = nc.dram_tensor("wt", [P, KC * D_HID], f32r, kind="ExternalInput").ap()
    oh_d = nc.dram_tensor("oh", [P, TILES * NUM_CLUSTERS], f32r, kind="ExternalInput").ap()
    bb_d = nc.dram_tensor("bb", [P, D_HID], f32, kind="ExternalInput").ap()
    sums_d = nc.dram_tensor("sums", [NUM_CLUSTERS, D_HID], f32, kind="ExternalOutput").ap()

    with tile.TileContext(nc) as tc:
        with (
            tc.tile_pool(name="consts", bufs=1) as cpool,
            tc.tile_pool(name="xt", bufs=2) as xpool,
            tc.tile_pool(name="phi", bufs=4) as fpool,
            tc.tile_pool(name="psum", bufs=4, space="PSUM") as ppool,
            tc.tile_pool(name="acc", bufs=1, space="PSUM") as apool,
        ):
            wt_sb = cpool.tile([P, KC * D_HID], f32r)
            nc.sync.dma_start(out=wt_sb, in_=wt_d)
            oh_sb = cpool.tile([P, TILES * NUM_CLUSTERS], f32r)
            nc.sync.dma_start(out=oh_sb, in_=oh_d)
            bb_sb = cpool.tile([P, D_HID], f32)
            nc.sync.dma_start(out=bb_sb, in_=bb_d)

            sums_ps = apool.tile([NUM_CLUSTERS, D_HID], f32)

            for blk in range(BLOCKS):
                xt_sb = xpool.tile([P, XCOLS], f32r)
                nc.sync.dma_start(out=xt_sb, in_=xt_d[:, blk * XCOLS:(blk + 1) * XCOLS])
                for j in range(TPB):
                    t = blk * TPB + j
                    phi_ps = ppool.tile([P, D_HID], f32)
                    for k in range(KC):
                        nc.tensor.matmul(
                            phi_ps,
                            xt_sb[:, k * BLKN + j * P: k * BLKN + (j + 1) * P],
                            wt_sb[:, k * D_HID:(k + 1) * D_HID],
                            start=(k == 0),
                            stop=(k == KC - 1),
                        )
                    tmp_sb = fpool.tile([P, D_HID], f32, tag="tmp")
                    nc.vector.tensor_add(tmp_sb, phi_ps, bb_sb)
                    phi_sb = fpool.tile([P, D_HID], f32r, tag="phi")
                    nc.scalar.activation(phi_sb, tmp_sb, mybir.ActivationFunctionType.Relu)
                    nc.tensor.matmul(
                        sums_ps,
                        oh_sb[:, t * NUM_CLUSTERS:(t + 1) * NUM_CLUSTERS],
                        phi_sb,
                        start=(t == 0),
                        stop=(t == TILES - 1),
                    )

            out_sb = cpool.tile([NUM_CLUSTERS, D_HID], f32)
            nc.vector.tensor_copy(out_sb, sums_ps)
            nc.sync.dma_start(out=sums_d, in_=out_sb)

    nc.compile()
    _CACHE["nc"] = nc
    return nc


def _round_f32r(a):
    """In-place round fp32 -> fp32r (round-to-nearest at 12 low mantissa bits)."""
    u = a.view(np.uint32)
    u += 0x800
    u &= 0xFFFFF000
    return a


def _shard_layout(x2, win):
    """Device X layout for one core: [P, BLOCKS*KC*BLKN] where
    out[p, (b*KC + kc)*BLKN + n] = x2[win + b*BLKN + n, kc*P + p]."""
    xw = x2[win:win + NP]                                  # [NP, D_IN]
    xr = xw.reshape(BLOCKS, BLKN, KC, P)
    out = np.empty((P, BLOCKS, KC, BLKN), np.float32)
    for b in range(BLOCKS):                                # blocked for cache
        out[:, b] = xr[b].transpose(2, 1, 0)               # [P, KC, BLKN]
    return out.reshape(P, BLOCKS * XCOLS)


def _prepare_in_maps(X, cluster_id, W_phi, b_phi):
    x2 = _round_f32r(np.array(np.asarray(X, np.float32)[0], copy=True))
    cid = np.asarray(cluster_id).astype(np.int64)

    onehot = (cid[:, None] == np.arange(NUM_CLUSTERS)[None, :]).astype(np.float32)

    wp = np.asarray(W_phi, np.float32)                     # [256, 1024]
    # wt[p, kc*256 + d] = W_phi[d, kc*128 + p]
    wt = _round_f32r(
        np.ascontiguousarray(wp.T.reshape(KC, P, D_HID).transpose(1, 0, 2).reshape(P, KC * D_HID))
    )
    bbc = np.ascontiguousarray(
        np.broadcast_to(np.asarray(b_phi, np.float32)[None, :], (P, D_HID))
    )

    in_maps = []
    for c in range(NCORES):
        win = min(c * OWN, N - NP)          # window start (core 7 shifts back)
        own_lo = c * OWN - win              # owned range within window
        own_hi = own_lo + OWN
        xt = _shard_layout(x2, win)
        oh = np.zeros((NP, NUM_CLUSTERS), np.float32)
        oh[own_lo:own_hi] = onehot[win + own_lo: win + own_hi]
        # device layout: [128, TILES*10], entry [p, t*10+c] = onehot[t*128+p, c]
        ohr = np.ascontiguousarray(
            oh.reshape(TILES, P, NUM_CLUSTERS).transpose(1, 0, 2).reshape(P, TILES * NUM_CLUSTERS)
        )
        in_maps.append({"xt": xt, "wt": wt, "oh": ohr, "bb": bbc})
    return in_maps


def kernel(X, cluster_id, W_phi, b_phi, W1, b1, Wa, ba, Wb, bb, Wc, bc, Wo, bo):
    cid = np.asarray(cluster_id).astype(np.int64)
    in_maps = _prepare_in_maps(X, cluster_id, W_phi, b_phi)

    nc = _build()
    res = run_bass_kernel_spmd(nc, in_maps, list(range(NCORES)))

    sums = np.zeros((NUM_CLUSTERS, D_HID), np.float32)
    for c in range(NCORES):
        sums += res.results[c]["sums"]

    counts = np.bincount(cid, minlength=NUM_CLUSTERS).astype(np.float32)

    # tiny attention-pooling + output head, fp32 on host (matches reference)
    h = np.where(counts[:, None] > 0, sums / np.maximum(counts, 1.0)[:, None], 0.0).astype(np.float32)
    h1 = np.maximum(h @ np.asarray(W1, np.float32).T + b1, 0.0).astype(np.float32)
    a = np.tanh(h1 @ np.asarray(Wa, np.float32).T + ba).astype(np.float32)
    g = (1.0 / (1.0 + np.exp(-(h1 @ np.asarray(Wb, np.float32).T + bb)))).astype(np.float32)
    scores = ((a * g) @ np.asarray(Wc, np.float32).T + bc).astype(np.float32)  # [10, 1]
    s = scores.T  # [1, 10]
    e = np.exp(s - s.max(axis=-1, keepdims=True))
    A = (e / e.sum(axis=-1, keepdims=True)).astype(np.float32)
    H = (A @ h1).astype(np.float32)
    out = (H @ np.asarray(Wo, np.float32).T + bo).astype(np.float32)
    return out


# revision 15
# speedup vs baseline: 2.9980x; 1.1885x over previous
"""DeepAttnMISL segment-reduce kernel for 8 TRN2 NeuronCores.

Strategy (per sharding hint): shard the N=200000 patches across the 8 cores.
Each core computes phi = relu(X_shard @ W_phi.T + b_phi) and per-cluster
partial sums via a one-hot matmul accumulated in PSUM. The tiny [10, 256]
per-core partial sums are reduced on the host, and the (tiny) attention
pooling + output head run on the host in fp32.

Device-side layout choices:
  - The d_in contraction dim must land on SBUF partitions, so the host
    pre-permutes X into [128, blocks*kc*blkn] per core (fp32 hardware
    DMA-transpose is not supported, and multi-chunk DMA APs hang TRN2, so
    every DMA here is a plain 2D row-contiguous transfer).
  - Each core gets a 25088-row (196x128) window of X; windows overlap so all
    cores run the identical SPMD program. The host-built one-hot matrix is
    zeroed outside each core's owned range, so overlap rows contribute
    nothing to the cluster sums.
  - Cluster counts depend only on cluster_id and are computed on the host.
"""

import numpy as np

import concourse.mybir as mybir
import concourse.tile as tile
from concourse import bacc
from concourse.bass_utils import run_bass_kernel_spmd

N = 200000
D_IN = 1024
D_HID = 256
NUM_CLUSTERS = 10
NCORES = 8
P = 128
KC = D_IN // P          # 8 contraction chunks
NP = 25088              # padded patches per core (196 * 128)
TILES = NP // P         # 196 patch tiles per core
TPB = 7                 # patch tiles per DMA block
BLOCKS = TILES // TPB   # 28 blocks
BLKN = TPB * P          # 1792 patches per block
OWN = N // NCORES       # 25000 owned rows per core
XCOLS = KC * BLKN       # free-dim columns per block in the device X layout

_CACHE = {}


def _build():
    if "nc" in _CACHE:
        return _CACHE["nc"]
    f32 = mybir.dt.float32
    f32r = mybir.dt.float32r
    nc = bacc.Bacc("TRN2", target_bir_lowering=False, debug=False, num_devices=NCORES)

    xt_d = nc.dram_tensor("xt", [P, BLOCKS * XCOLS], f32r, kind="ExternalInput").ap()
    wt_d = nc.dram_tensor("wt", [P, KC * D_HID], f32r, kind="ExternalInput").ap()
    oh_d = nc.dram_tensor("oh", [P, TILES * NUM_CLUSTERS], f32r, kind="ExternalInput").ap()
    bb_d = nc.dram_tensor("bb", [P, D_HID], f32, kind="ExternalInput").ap()
    sums_d = nc.dram_tensor("sums", [NUM_CLUSTERS, D_HID], f32, kind="ExternalOutput").ap()

    with tile.TileContext(nc) as tc:
        with (
            tc.tile_pool(name="consts", bufs=1) as cpool,
            tc.tile_pool(name="xt", bufs=4) as xpool,
            tc.tile_pool(name="phi", bufs=4) as fpool,
            tc.tile_pool(name="psum", bufs=4, space="PSUM") as ppool,
            tc.tile_pool(name="acc", bufs=1, space="PSUM") as apool,
        ):
            wt_sb = cpool.tile([P, KC * D_HID], f32r)
            nc.sync.dma_start(out=wt_sb, in_=wt_d)
            oh_sb = cpool.tile([P, TILES * NUM_CLUSTERS], f32r)
            nc.sync.dma_start(out=oh_sb, in_=oh_d)
            bb_sb = cpool.tile([P, D_HID], f32)
            nc.sync.dma_start(out=bb_sb, in_=bb_d)

            sums_ps = apool.tile([NUM_CLUSTERS, D_HID], f32)

            for blk in range(BLOCKS):
                xt_sb = xpool.tile([P, XCOLS], f32r)
                nc.sync.dma_start(out=xt_sb, in_=xt_d[:, blk * XCOLS:(blk + 1) * XCOLS])
                for j in range(TPB):
                    t = blk * TPB + j
                    phi_ps = ppool.tile([P, D_HID], f32)
                    for k in range(KC):
                        nc.tensor.matmul(
                            phi_ps,
                            xt_sb[:, (j * KC + k) * P:(j * KC + k + 1) * P],
                            wt_sb[:, k * D_HID:(k + 1) * D_HID],
                            start=(k == 0),
                            stop=(k == KC - 1),
                        )
                    tmp_sb = fpool.tile([P, D_HID], f32, tag="tmp")
                    nc.vector.tensor_add(tmp_sb, phi_ps, bb_sb)
                    phi_sb = fpool.tile([P, D_HID], f32r, tag="phi")
                    nc.scalar.activation(phi_sb, tmp_sb, mybir.ActivationFunctionType.Relu)
                    nc.tensor.matmul(
                        sums_ps,
                        oh_sb[:, t * NUM_CLUSTERS:(t + 1) * NUM_CLUSTERS],
                        phi_sb,
                        start=(t == 0),
                        stop=(t == TILES - 1),
                    )

            out_sb = cpool.tile([NUM_CLUSTERS, D_HID], f32)
            nc.vector.tensor_copy(out_sb, sums_ps)
            nc.sync.dma_start(out=sums_d, in_=out_sb)

    nc.compile()
    _CACHE["nc"] = nc
    return nc


def _round_f32r(a):
    """In-place round fp32 -> fp32r (round-to-nearest at 12 low mantissa bits)."""
    u = a.view(np.uint32)
    u += 0x800
    u &= 0xFFFFF000
    return a


def _shard_layout(x2, win):
    """Device X layout for one core: [P, TILES*KC*P] where
    out[p, (t*KC + k)*P + n] = x2[win + t*P + n, k*P + p]."""
    xr = x2[win:win + NP].reshape(TILES, P, KC, P)         # [t, n, k, p]
    out = np.empty((P, TILES, KC, P), np.float32)
    for t in range(TILES):                                 # blocked for cache
        out[:, t] = xr[t].transpose(2, 1, 0)               # [p, k, n]
    return out.reshape(P, TILES * KC * P)


def _prepare_in_maps(X, cluster_id, W_phi, b_phi):
    x2 = _round_f32r(np.array(np.asarray(X, np.float32)[0], copy=True))
    cid = np.asarray(cluster_id).astype(np.int64)

    onehot = (cid[:, None] == np.arange(NUM_CLUSTERS)[None, :]).astype(np.float32)

    wp = np.asarray(W_phi, np.float32)                     # [256, 1024]
    # wt[p, kc*256 + d] = W_phi[d, kc*128 + p]
    wt = _round_f32r(
        np.ascontiguousarray(wp.T.reshape(KC, P, D_HID).transpose(1, 0, 2).reshape(P, KC * D_HID))
    )
    bbc = np.ascontiguousarray(
        np.broadcast_to(np.asarray(b_phi, np.float32)[None, :], (P, D_HID))
    )

    in_maps = []
    for c in range(NCORES):
        win = min(c * OWN, N - NP)          # window start (core 7 shifts back)
        own_lo = c * OWN - win              # owned range within window
        own_hi = own_lo + OWN
        xt = _shard_layout(x2, win)
        oh = np.zeros((NP, NUM_CLUSTERS), np.float32)
        oh[own_lo:own_hi] = onehot[win + own_lo: win + own_hi]
        # device layout: [128, TILES*10], entry [p, t*10+c] = onehot[t*128+p, c]
        ohr = np.ascontiguousarray(
            oh.reshape(TILES, P, NUM_CLUSTERS).transpose(1, 0, 2).reshape(P, TILES * NUM_CLUSTERS)
        )
        in_maps.append({"xt": xt, "wt": wt, "oh": ohr, "bb": bbc})
    return in_maps


def kernel(X, cluster_id, W_phi, b_phi, W1, b1, Wa, ba, Wb, bb, Wc, bc, Wo, bo):
    cid = np.asarray(cluster_id).astype(np.int64)
    in_maps = _prepare_in_maps(X, cluster_id, W_phi, b_phi)

    nc = _build()
    res = run_bass_kernel_spmd(nc, in_maps, list(range(NCORES)))

    sums = np.zeros((NUM_CLUSTERS, D_HID), np.float32)
    for c in range(NCORES):
        sums += res.results[c]["sums"]

    counts = np.bincount(cid, minlength=NUM_CLUSTERS).astype(np.float32)

    # tiny attention-pooling + output head, fp32 on host (matches reference)
    h = np.where(counts[:, None] > 0, sums / np.maximum(counts, 1.0)[:, None], 0.0).astype(np.float32)
    h1 = np.maximum(h @ np.asarray(W1, np.float32).T + b1, 0.0).astype(np.float32)
    a = np.tanh(h1 @ np.asarray(Wa, np.float32).T + ba).astype(np.float32)
    g = (1.0 / (1.0 + np.exp(-(h1 @ np.asarray(Wb, np.float32).T + bb)))).astype(np.float32)
    scores = ((a * g) @ np.asarray(Wc, np.float32).T + bc).astype(np.float32)  # [10, 1]
    s = scores.T  # [1, 10]
    e = np.exp(s - s.max(axis=-1, keepdims=True))
    A = (e / e.sum(axis=-1, keepdims=True)).astype(np.float32)
    H = (A @ h1).astype(np.float32)
    out = (H @ np.asarray(Wo, np.float32).T + bo).astype(np.float32)
    return out
